# revision 5
# baseline (speedup 1.0000x reference)
"""Trainium2 Bass kernel for BSGRUCell (block-sparse GRU cell with soft MoE routing).

Math (algebraically equivalent to the reference, verified):
    ky  = softmax(BETA * (input @ w_ik + hx @ w_hk + b_ik))            [B, K]
    G_i = input @ W2d.T          (W2d = weight_ih.reshape(3H, IN))     [B, 3H]
    G_h = hx    @ U2d.T          (U2d = weight_hh.reshape(3H, H))      [B, 3H]
    i_gates = G_i * ky[:, o//768]                       + bias_ih
    h_gates = G_h * kx[:, o//768] * ky[:, (o%768)//192] + bias_hh
    z, r = sigmoid(i_z + h_z), sigmoid(i_r + h_r)
    n    = tanh(i_n + h_n * r)
    hy   = n * ky_h + z * (hx * kx_h - n * ky_h),  ky_h/kx_h = col h//256

Sharding: the 3H=3072 gate-output rows are split by hidden slice — core m owns
hidden columns [m*128, (m+1)*128) and the three gate rows g*1024 + m*128 for
g in z,r,n.  No cross-core communication; weights are read exactly once across
the 8 cores (memory-roofline sharding).  All per-core index variation (which
ky/kx columns scale which output segments) is passed as small one-hot selection
matrices so a single SPMD program serves all cores.

Device-side structure per core:
  - small data-independent warmup matmuls get the PE HAM clock to 8/8 before
    the real fp32 matmuls arrive (fp32 is two PE passes per matmul)
  - BETA is folded into the routing weights on the host; the routing bias is
    added via a K=1 ones-row matmul so the softmax needs no pre-add
  - sigmoid(x) is computed as 0.5*tanh(0.5x)+0.5 so exp/tanh share one ACT
    table set; the 0.5/2.0 factors are folded into the one-hot gather matrices
    and biases on the host, so no extra affine ops appear on the critical tail
  - U streams in PE-consumption order (r|n gate columns first, z last) so the
    post-DMA tail is short; the z gate only feeds the final two ops
"""

import numpy as np

B = 32
IN = 512
H = 1024
K = 4
BS = H // K          # 256
H3 = 3 * H           # 3072
NCORES = 8
HS = H // NCORES     # 128
BETA = 10.0

# misc32 column layout
_M32_HX = 0          # [0:128)    hx[:, m*128:(m+1)*128]
_M32_BSZ = 128       # [128:256)  (bias_ih+bias_hh) z-slice, tiled to 32 rows
_M32_BSR = 256       # [256:384)  (bias_ih+bias_hh) r-slice
_M32_BIN = 384       # [384:512)  bias_ih n-slice
_M32_BHN = 512       # [512:640)  0.5 * bias_hh n-slice
_M32_BIK = 640       # [640:644)  BETA * bias_ik tiled
_M32_W = 644

# misc128 column layout
_M128_INPT = 0       # [0:128)    input.T packed   [p, c4*32+b]
_M128_HXT = 128      # [128:384)  hx.T packed      [p, c8*32+b]
_M128_WIK = 384      # [384:400)  BETA*weight_ik packed [p, c4*4+k]
_M128_WHK = 400      # [400:432)  BETA*weight_hk packed [p, c8*4+k]
_M128_W = 432

# sel column layout: 10 ky one-hots, 4 kx one-hots, kx.T
# ky cols: [ki_z, ki_r, ki_n, k2_z0, k2_z1, k2_r0, k2_r1, k2_n0*, k2_n1*, kh*]
# kx cols: [k1_z, k1_r, k1_n, kh*]        (* = scaled by 0.5, see module doc)
_SEL_KY = 0          # [0:10)
_SEL_KX = 10         # [10:14)
_SEL_KXT = 14        # [14:46)
_SEL_W = 46

_N_WARMUP = 10


def _pack_rows(x, nchunk):
    """[nchunk*128, F] -> [128, nchunk*F] with col block c = rows [c*128,(c+1)*128)."""
    f = x.shape[1]
    return np.ascontiguousarray(
        x.reshape(nchunk, 128, f).transpose(1, 0, 2).reshape(128, nchunk * f)
    )


def prep_inputs(input, hx, kx, weight_ik, weight_hk, weight_ih, weight_hh,
                bias_ik, bias_ih, bias_hh):
    """Build the 8 per-core input maps (all float32 numpy)."""
    f32 = np.float32
    input = np.asarray(input, f32)
    hx = np.asarray(hx, f32)
    kx = np.asarray(kx, f32)
    weight_ik = np.asarray(weight_ik, f32)
    weight_hk = np.asarray(weight_hk, f32)
    W2d = np.asarray(weight_ih, f32).reshape(H3, IN)
    U2d = np.asarray(weight_hh, f32).reshape(H3, H)
    bias_ik = np.asarray(bias_ik, f32)
    bias_ih = np.asarray(bias_ih, f32)
    bias_hh = np.asarray(bias_hh, f32)
    bsum = bias_ih + bias_hh

    misc128 = np.empty((128, _M128_W), f32)
    misc128[:, _M128_INPT:_M128_INPT + 128] = _pack_rows(input.T, 4)
    misc128[:, _M128_HXT:_M128_HXT + 256] = _pack_rows(hx.T, 8)
    misc128[:, _M128_WIK:_M128_WIK + 16] = _pack_rows(BETA * weight_ik, 4)
    misc128[:, _M128_WHK:_M128_WHK + 32] = _pack_rows(BETA * weight_hk, 8)

    in_maps = []
    for m in range(NCORES):
        og = [g * H + m * HS for g in range(3)]

        misc32 = np.empty((B, _M32_W), f32)
        misc32[:, _M32_HX:_M32_HX + HS] = hx[:, m * HS:(m + 1) * HS]
        misc32[:, _M32_BSZ:_M32_BSZ + HS] = bsum[og[0]:og[0] + HS][None, :]
        misc32[:, _M32_BSR:_M32_BSR + HS] = bsum[og[1]:og[1] + HS][None, :]
        misc32[:, _M32_BIN:_M32_BIN + HS] = bias_ih[og[2]:og[2] + HS][None, :]
        misc32[:, _M32_BHN:_M32_BHN + HS] = 0.5 * bias_hh[og[2]:og[2] + HS][None, :]
        misc32[:, _M32_BIK:_M32_BIK + K] = BETA * bias_ik[None, :]

        sel = np.zeros((K, _SEL_W), f32)
        # ky gathers; n-gate k2 entries and kh carry the folded 0.5
        idx_ky = [og[0] // 768, og[1] // 768, og[2] // 768]
        for g in range(3):
            a = og[g] % 768
            idx_ky += [a // 192, (a + 64) // 192]
        idx_ky.append(m // 2)
        for j, k in enumerate(idx_ky):
            sel[k, _SEL_KY + j] = 0.5 if j in (7, 8, 9) else 1.0
        # kx gathers: [k1_z, k1_r, k1_n, 0.5*kh]
        idx_kx = [og[0] // 768, og[1] // 768, og[2] // 768, m // 2]
        for j, k in enumerate(idx_kx):
            sel[k, _SEL_KX + j] = 0.5 if j == 3 else 1.0
        sel[:, _SEL_KXT:_SEL_KXT + B] = kx.T

        rows = np.concatenate([np.arange(o, o + HS) for o in og])
        w = _pack_rows(np.ascontiguousarray(W2d[rows].T), 4)        # [128, 1536]
        Um = U2d[rows]                                              # [384, 1024]
        u_rn = _pack_rows(np.ascontiguousarray(Um[HS:3 * HS].T), 8)  # [128, 2048]
        u_z = _pack_rows(np.ascontiguousarray(Um[0:HS].T), 8)        # [128, 1024]

        in_maps.append({
            "misc128": misc128, "misc32": misc32, "sel": sel, "w": w,
            "u_rn": u_rn, "u_z": u_z,
        })
    return in_maps


def build_nc():
    import concourse.bacc as bacc
    import concourse.bass as bass
    import concourse.tile as tile
    from concourse import mybir

    dt = mybir.dt.float32
    AF = mybir.ActivationFunctionType
    OP = mybir.AluOpType
    AX = mybir.AxisListType

    nc = bacc.Bacc("TRN2")
    d_misc128 = nc.dram_tensor("misc128", [128, _M128_W], dt, kind="ExternalInput")
    d_misc32 = nc.dram_tensor("misc32", [B, _M32_W], dt, kind="ExternalInput")
    d_sel = nc.dram_tensor("sel", [K, _SEL_W], dt, kind="ExternalInput")
    d_w = nc.dram_tensor("w", [128, 1536], dt, kind="ExternalInput")
    d_u_rn = nc.dram_tensor("u_rn", [128, 2048], dt, kind="ExternalInput")
    d_u_z = nc.dram_tensor("u_z", [128, 1024], dt, kind="ExternalInput")
    d_out = nc.dram_tensor("out", [B, HS + K], dt, kind="ExternalOutput")

    with tile.TileContext(nc) as tc:
        with (
            tc.tile_pool(name="sb", bufs=1) as sb,
            tc.tile_pool(name="ps", bufs=1, space=bass.MemorySpace.PSUM) as ps,
        ):
            t_misc128 = sb.tile([128, _M128_W], dt, tag="misc128")
            t_misc32 = sb.tile([B, _M32_W], dt, tag="misc32")
            t_sel = sb.tile([K, _SEL_W], dt, tag="sel")
            t_w = sb.tile([128, 1536], dt, tag="w")
            t_urn = sb.tile([128, 2048], dt, tag="u_rn")
            t_uz = sb.tile([128, 1024], dt, tag="u_z")
            # warmup tiles are zeroed first on GpSimd (earliest-ready queue),
            # then misc128 streams via SWDGE while the SP-queue preamble is
            # still draining; weights stream on the SP queue split into
            # pieces matching PE consumption order so each piece's semaphore
            # releases its matmuls as early as possible
            t_wl = sb.tile([128, B], dt, tag="warm_l")
            t_wr = sb.tile([128, 64], dt, tag="warm_r")
            nc.gpsimd.memset(t_wl[:], 0.0)
            nc.gpsimd.memset(t_wr[:], 0.0)
            nc.gpsimd.dma_start(t_misc128[:], d_misc128[:])
            nc.sync.dma_start(t_w[:, 0:768], d_w[:, 0:768])
            nc.sync.dma_start(t_w[:, 768:1536], d_w[:, 768:1536])
            nc.sync.dma_start(t_urn[:, 0:1024], d_u_rn[:, 0:1024])
            nc.sync.dma_start(t_urn[:, 1024:2048], d_u_rn[:, 1024:2048])
            nc.sync.dma_start(t_uz[:, 0:768], d_u_z[:, 0:768])
            nc.sync.dma_start(t_uz[:, 768:1024], d_u_z[:, 768:1024])
            nc.scalar.dma_start(t_misc32[:], d_misc32[:])
            nc.scalar.dma_start(t_sel[:], d_sel[:])

            def inpT(c4):
                return t_misc128[:, _M128_INPT + c4 * B:_M128_INPT + (c4 + 1) * B]

            def hxT(c8):
                return t_misc128[:, _M128_HXT + c8 * B:_M128_HXT + (c8 + 1) * B]

            # PE warmup: small data-independent matmuls fill the PE-idle
            # window before routing so the HAM clock ramps early
            t_ones = sb.tile([1, B], dt, tag="ones")
            nc.gpsimd.memset(t_ones[:], 1.0)
            ps_warm = ps.tile([B, 64], dt, tag="ps_warm")
            for _ in range(_N_WARMUP):
                nc.tensor.matmul(ps_warm[:], t_wl[:], t_wr[:], start=True, stop=True)

            # routing logits: BETA*(input @ w_ik + hx @ w_hk + b_ik) -> [B, K]
            # (BETA folded into the weights host-side; bias via ones-row matmul)
            ps_ky = ps.tile([B, K], dt, tag="ps_ky")
            for c4 in range(4):
                nc.tensor.matmul(
                    ps_ky[:], inpT(c4),
                    t_misc128[:, _M128_WIK + c4 * K:_M128_WIK + (c4 + 1) * K],
                    start=(c4 == 0), stop=False)
            for c8 in range(8):
                nc.tensor.matmul(
                    ps_ky[:], hxT(c8),
                    t_misc128[:, _M128_WHK + c8 * K:_M128_WHK + (c8 + 1) * K],
                    start=False, stop=False)
            nc.tensor.matmul(ps_ky[:], t_ones[:],
                             t_misc32[0:1, _M32_BIK:_M32_BIK + K],
                             start=False, stop=True)

            # softmax over the K=4 free axis, normalized result lands in the
            # zero-padded [32, 32] tile that feeds the transpose
            t_kypad = sb.tile([B, B], dt, tag="kypad")
            nc.gpsimd.memset(t_kypad[:], 0.0)
            t_nmx = sb.tile([B, 1], dt, tag="nmx")
            nc.vector.reduce_max(t_nmx[:], ps_ky[:], axis=AX.X, negate=True)
            t_e = sb.tile([B, K], dt, tag="e")
            nc.scalar.activation(t_e[:], ps_ky[:], AF.Exp, bias=t_nmx[:])
            t_s = sb.tile([B, 1], dt, tag="s")
            nc.vector.reduce_sum(t_s[:], t_e[:], axis=AX.X)
            t_rs = sb.tile([B, 1], dt, tag="rs")
            nc.vector.reciprocal(t_rs[:], t_s[:])
            nc.vector.tensor_scalar_mul(t_kypad[:, 0:K], t_e[:], t_rs[:])
            t_kyT = sb.tile([B, B], dt, tag="kyT")
            nc.vector.transpose(t_kyT[:], t_kypad[:])

            # ky lands in the output tile early (off the critical tail)
            t_out = sb.tile([B, HS + K], dt, tag="out")
            nc.vector.tensor_copy(t_out[:, HS:HS + K], t_kypad[:, 0:K])

            # i-path: G_i = input @ Wm.T  -> [B, 384]
            ps_i = ps.tile([B, 3 * HS], dt, tag="ps_i")
            for c4 in range(4):
                nc.tensor.matmul(ps_i[:], inpT(c4),
                                 t_w[:, c4 * 384:(c4 + 1) * 384],
                                 start=(c4 == 0), stop=(c4 == 3))

            # gather the needed ky / kx columns via one-hot matmuls
            ps_g = ps.tile([B, 10], dt, tag="ps_g")
            nc.tensor.matmul(ps_g[:], t_kyT[0:K, :],
                             t_sel[0:K, _SEL_KY:_SEL_KY + 10], start=True, stop=True)
            ps_gx = ps.tile([B, 4], dt, tag="ps_gx")
            nc.tensor.matmul(ps_gx[:], t_sel[0:K, _SEL_KXT:_SEL_KXT + B],
                             t_sel[0:K, _SEL_KX:_SEL_KX + 4], start=True, stop=True)
            t_g = sb.tile([B, 14], dt, tag="g")
            nc.vector.tensor_copy(t_g[:, 0:10], ps_g[:])
            nc.vector.tensor_copy(t_g[:, 10:14], ps_gx[:])

            # kk products per 64-wide segment: kx[:,k1(g)] * ky[:,k2(g,seg)]
            # (n-gate entries carry the folded 0.5 via sel)
            t_kkp = sb.tile([B, 6], dt, tag="kkp")
            for j in range(6):
                g = j // 2
                nc.vector.tensor_mul(t_kkp[:, j:j + 1], t_g[:, 3 + j:4 + j],
                                     t_g[:, 10 + g:11 + g])
            # 0.5 * hx * kx_h, needed only at the very end — compute early
            t_u2h = sb.tile([B, HS], dt, tag="u2h")
            nc.vector.tensor_scalar_mul(t_u2h[:], t_misc32[:, _M32_HX:_M32_HX + HS],
                                        t_g[:, 13:14])

            # ib = ky_g * G_i + bias   (bias = bsum for z/r, bias_ih for n)
            t_ib = sb.tile([B, 3 * HS], dt, tag="ib")
            for g, boff in ((0, _M32_BSZ), (1, _M32_BSR), (2, _M32_BIN)):
                nc.vector.scalar_tensor_tensor(
                    t_ib[:, g * HS:(g + 1) * HS], ps_i[:, g * HS:(g + 1) * HS],
                    t_g[:, g:g + 1], t_misc32[:, boff:boff + HS],
                    op0=OP.mult, op1=OP.add)

            # h-path: r|n gates first (they gate the long tail), z last
            ps_rn = ps.tile([B, 2 * HS], dt, tag="ps_rn")
            for c8 in range(8):
                nc.tensor.matmul(ps_rn[:], hxT(c8),
                                 t_urn[:, c8 * 256:(c8 + 1) * 256],
                                 start=(c8 == 0), stop=(c8 == 7))
            ps_z = ps.tile([B, HS], dt, tag="ps_z")
            for c8 in range(8):
                nc.tensor.matmul(ps_z[:], hxT(c8),
                                 t_uz[:, c8 * HS:(c8 + 1) * HS],
                                 start=(c8 == 0), stop=(c8 == 7))

            # S = kk * G_h + ib per 64-segment
            t_Sr = sb.tile([B, HS], dt, tag="Sr")
            t_hn = sb.tile([B, HS], dt, tag="hn")   # 0.5*(kk*G_h_n + bias_hh_n)
            t_Sz = sb.tile([B, HS], dt, tag="Sz")
            for si in range(2):
                s0 = si * 64
                nc.vector.scalar_tensor_tensor(
                    t_Sr[:, s0:s0 + 64], ps_rn[:, s0:s0 + 64],
                    t_kkp[:, 2 + si:3 + si], t_ib[:, HS + s0:HS + s0 + 64],
                    op0=OP.mult, op1=OP.add)
            for si in range(2):
                s0 = si * 64
                nc.vector.scalar_tensor_tensor(
                    t_hn[:, s0:s0 + 64], ps_rn[:, HS + s0:HS + s0 + 64],
                    t_kkp[:, 4 + si:5 + si],
                    t_misc32[:, _M32_BHN + s0:_M32_BHN + s0 + 64],
                    op0=OP.mult, op1=OP.add)

            # sigmoid via tanh: r = 0.5*th_r + 0.5, and r*h_n = (th_r+1)*hn
            t_thr = sb.tile([B, HS], dt, tag="thr")
            nc.scalar.activation(t_thr[:], t_Sr[:], AF.Tanh, scale=0.5)
            t_hnr = sb.tile([B, HS], dt, tag="hnr")
            nc.vector.scalar_tensor_tensor(t_hnr[:], t_thr[:], 1.0, t_hn[:],
                                           op0=OP.add, op1=OP.mult)
            t_pren = sb.tile([B, HS], dt, tag="pren")
            nc.vector.tensor_add(t_pren[:], t_ib[:, 2 * HS:3 * HS], t_hnr[:])
            t_newg = sb.tile([B, HS], dt, tag="newg")
            nc.scalar.activation(t_newg[:], t_pren[:], AF.Tanh)

            for si in range(2):
                s0 = si * 64
                nc.vector.scalar_tensor_tensor(
                    t_Sz[:, s0:s0 + 64], ps_z[:, s0:s0 + 64],
                    t_kkp[:, si:si + 1], t_ib[:, s0:s0 + 64],
                    op0=OP.mult, op1=OP.add)
            t_thz = sb.tile([B, HS], dt, tag="thz")
            nc.scalar.activation(t_thz[:], t_Sz[:], AF.Tanh, scale=0.5)

            # hy = 2*t1h + (th_z+1)*d0h,  t1h = 0.5*n*ky_h,  d0h = u2h - t1h
            t_t1h = sb.tile([B, HS], dt, tag="t1h")
            nc.vector.tensor_scalar_mul(t_t1h[:], t_newg[:], t_g[:, 9:10])
            t_d0h = sb.tile([B, HS], dt, tag="d0h")
            nc.vector.tensor_sub(t_d0h[:], t_u2h[:], t_t1h[:])
            t_e2 = sb.tile([B, HS], dt, tag="e2")
            nc.vector.scalar_tensor_tensor(t_e2[:], t_thz[:], 1.0, t_d0h[:],
                                           op0=OP.add, op1=OP.mult)
            nc.vector.scalar_tensor_tensor(t_out[:, 0:HS], t_t1h[:], 2.0, t_e2[:],
                                           op0=OP.mult, op1=OP.add)
            nc.sync.dma_start(d_out[:], t_out[:])

    nc.finalize()
    return nc


_NC = None


def _get_nc():
    global _NC
    if _NC is None:
        _NC = build_nc()
    return _NC


def kernel(**inputs):
    from concourse.bass_utils import run_bass_kernel_spmd

    nc = _get_nc()
    in_maps = prep_inputs(**{k: np.asarray(v) for k, v in inputs.items()})
    res = run_bass_kernel_spmd(nc, in_maps, list(range(NCORES)))
    outs = res.results
    hy = np.concatenate([outs[m]["out"][:, 0:HS] for m in range(NCORES)], axis=1)
    ky = np.ascontiguousarray(outs[0]["out"][:, HS:HS + K])
    return hy, ky


# revision 6
# speedup vs baseline: 1.0621x; 1.0621x over previous
"""Trainium2 Bass kernel for BSGRUCell (block-sparse GRU cell with soft MoE routing).

Math (algebraically equivalent to the reference, verified):
    ky  = softmax(BETA * (input @ w_ik + hx @ w_hk + b_ik))            [B, K]
    G_i = input @ W2d.T          (W2d = weight_ih.reshape(3H, IN))     [B, 3H]
    G_h = hx    @ U2d.T          (U2d = weight_hh.reshape(3H, H))      [B, 3H]
    i_gates = G_i * ky[:, o//768]                       + bias_ih
    h_gates = G_h * kx[:, o//768] * ky[:, (o%768)//192] + bias_hh
    z, r = sigmoid(i_z + h_z), sigmoid(i_r + h_r)
    n    = tanh(i_n + h_n * r)
    hy   = n * ky_h + z * (hx * kx_h - n * ky_h),  ky_h/kx_h = col h//256

Sharding: the 3H=3072 gate-output rows are split by hidden slice — core m owns
hidden columns [m*128, (m+1)*128) and the three gate rows g*1024 + m*128 for
g in z,r,n.  No cross-core communication; weights are read exactly once across
the 8 cores (memory-roofline sharding).  All per-core index variation (which
ky/kx columns scale which output segments) is passed as small one-hot selection
matrices so a single SPMD program serves all cores.

Device-side structure per core:
  - small data-independent warmup matmuls get the PE HAM clock to 8/8 before
    the real fp32 matmuls arrive (fp32 is two PE passes per matmul)
  - BETA is folded into the routing weights on the host; the routing bias is
    added via a K=1 ones-row matmul so the softmax needs no pre-add
  - sigmoid(x) is computed as 0.5*tanh(0.5x)+0.5 so exp/tanh share one ACT
    table set; the 0.5/2.0 factors are folded into the one-hot gather matrices
    and biases on the host, so no extra affine ops appear on the critical tail
  - U streams in PE-consumption order (r|n gate columns first, z last) so the
    post-DMA tail is short; the z gate only feeds the final two ops
"""

import numpy as np

B = 32
IN = 512
H = 1024
K = 4
BS = H // K          # 256
H3 = 3 * H           # 3072
NCORES = 8
HS = H // NCORES     # 128
BETA = 10.0

# misc32 column layout
_M32_HX = 0          # [0:128)    hx[:, m*128:(m+1)*128]
_M32_BSZ = 128       # [128:256)  (bias_ih+bias_hh) z-slice, tiled to 32 rows
_M32_BSR = 256       # [256:384)  (bias_ih+bias_hh) r-slice
_M32_BIN = 384       # [384:512)  bias_ih n-slice
_M32_BHN = 512       # [512:640)  0.5 * bias_hh n-slice
_M32_BIK = 640       # [640:644)  BETA * bias_ik tiled
_M32_W = 644

# misc128 column layout
_M128_INPT = 0       # [0:128)    input.T packed   [p, c4*32+b]
_M128_HXT = 128      # [128:384)  hx.T packed      [p, c8*32+b]
_M128_WIK = 384      # [384:400)  BETA*weight_ik packed [p, c4*4+k]
_M128_WHK = 400      # [400:432)  BETA*weight_hk packed [p, c8*4+k]
_M128_W = 432

# sel column layout: 10 ky one-hots, 4 kx one-hots, kx.T
# ky cols: [ki_z, ki_r, ki_n, k2_z0, k2_z1, k2_r0, k2_r1, k2_n0*, k2_n1*, kh*]
# kx cols: [k1_z, k1_r, k1_n, kh*]        (* = scaled by 0.5, see module doc)
_SEL_KY = 0          # [0:10)
_SEL_KX = 10         # [10:14)
_SEL_KXT = 14        # [14:46)
_SEL_W = 46

_N_WARMUP = 10


def _pack_rows(x, nchunk):
    """[nchunk*128, F] -> [128, nchunk*F] with col block c = rows [c*128,(c+1)*128)."""
    f = x.shape[1]
    return np.ascontiguousarray(
        x.reshape(nchunk, 128, f).transpose(1, 0, 2).reshape(128, nchunk * f)
    )


def prep_inputs(input, hx, kx, weight_ik, weight_hk, weight_ih, weight_hh,
                bias_ik, bias_ih, bias_hh):
    """Build the 8 per-core input maps (all float32 numpy)."""
    f32 = np.float32
    input = np.asarray(input, f32)
    hx = np.asarray(hx, f32)
    kx = np.asarray(kx, f32)
    weight_ik = np.asarray(weight_ik, f32)
    weight_hk = np.asarray(weight_hk, f32)
    W2d = np.asarray(weight_ih, f32).reshape(H3, IN)
    U2d = np.asarray(weight_hh, f32).reshape(H3, H)
    bias_ik = np.asarray(bias_ik, f32)
    bias_ih = np.asarray(bias_ih, f32)
    bias_hh = np.asarray(bias_hh, f32)
    bsum = bias_ih + bias_hh

    misc128 = np.empty((128, _M128_W), f32)
    misc128[:, _M128_INPT:_M128_INPT + 128] = _pack_rows(input.T, 4)
    misc128[:, _M128_HXT:_M128_HXT + 256] = _pack_rows(hx.T, 8)
    misc128[:, _M128_WIK:_M128_WIK + 16] = _pack_rows(BETA * weight_ik, 4)
    misc128[:, _M128_WHK:_M128_WHK + 32] = _pack_rows(BETA * weight_hk, 8)

    in_maps = []
    for m in range(NCORES):
        og = [g * H + m * HS for g in range(3)]

        misc32 = np.empty((B, _M32_W), f32)
        misc32[:, _M32_HX:_M32_HX + HS] = hx[:, m * HS:(m + 1) * HS]
        misc32[:, _M32_BSZ:_M32_BSZ + HS] = bsum[og[0]:og[0] + HS][None, :]
        misc32[:, _M32_BSR:_M32_BSR + HS] = bsum[og[1]:og[1] + HS][None, :]
        misc32[:, _M32_BIN:_M32_BIN + HS] = bias_ih[og[2]:og[2] + HS][None, :]
        misc32[:, _M32_BHN:_M32_BHN + HS] = 0.5 * bias_hh[og[2]:og[2] + HS][None, :]
        misc32[:, _M32_BIK:_M32_BIK + K] = BETA * bias_ik[None, :]

        sel = np.zeros((K, _SEL_W), f32)
        # ky gathers; n-gate k2 entries and kh carry the folded 0.5
        idx_ky = [og[0] // 768, og[1] // 768, og[2] // 768]
        for g in range(3):
            a = og[g] % 768
            idx_ky += [a // 192, (a + 64) // 192]
        idx_ky.append(m // 2)
        for j, k in enumerate(idx_ky):
            sel[k, _SEL_KY + j] = 0.5 if j in (7, 8, 9) else 1.0
        # kx gathers: [k1_z, k1_r, k1_n, 0.5*kh]
        idx_kx = [og[0] // 768, og[1] // 768, og[2] // 768, m // 2]
        for j, k in enumerate(idx_kx):
            sel[k, _SEL_KX + j] = 0.5 if j == 3 else 1.0
        sel[:, _SEL_KXT:_SEL_KXT + B] = kx.T

        rows = np.concatenate([np.arange(o, o + HS) for o in og])
        w = _pack_rows(np.ascontiguousarray(W2d[rows].T), 4)        # [128, 1536]
        Um = U2d[rows]                                              # [384, 1024]
        u_rn = _pack_rows(np.ascontiguousarray(Um[HS:3 * HS].T), 8)  # [128, 2048]
        u_z = _pack_rows(np.ascontiguousarray(Um[0:HS].T), 8)        # [128, 1024]

        in_maps.append({
            "misc128": misc128, "misc32": misc32, "sel": sel, "w": w,
            "u_rn": u_rn, "u_z": u_z,
        })
    return in_maps


def build_nc():
    import concourse.bacc as bacc
    import concourse.bass as bass
    import concourse.tile as tile
    from concourse import mybir

    dt = mybir.dt.float32
    AF = mybir.ActivationFunctionType
    OP = mybir.AluOpType
    AX = mybir.AxisListType

    nc = bacc.Bacc("TRN2")
    d_misc128 = nc.dram_tensor("misc128", [128, _M128_W], dt, kind="ExternalInput")
    d_misc32 = nc.dram_tensor("misc32", [B, _M32_W], dt, kind="ExternalInput")
    d_sel = nc.dram_tensor("sel", [K, _SEL_W], dt, kind="ExternalInput")
    d_w = nc.dram_tensor("w", [128, 1536], dt, kind="ExternalInput")
    d_u_rn = nc.dram_tensor("u_rn", [128, 2048], dt, kind="ExternalInput")
    d_u_z = nc.dram_tensor("u_z", [128, 1024], dt, kind="ExternalInput")
    d_out = nc.dram_tensor("out", [B, HS + K], dt, kind="ExternalOutput")

    with tile.TileContext(nc) as tc:
        with (
            tc.tile_pool(name="sb", bufs=1) as sb,
            tc.tile_pool(name="ps", bufs=1, space=bass.MemorySpace.PSUM) as ps,
        ):
            t_misc128 = sb.tile([128, _M128_W], dt, tag="misc128")
            t_misc32 = sb.tile([B, _M32_W], dt, tag="misc32")
            t_sel = sb.tile([K, _SEL_W], dt, tag="sel")
            t_w = sb.tile([128, 1536], dt, tag="w")
            t_urn = sb.tile([128, 2048], dt, tag="u_rn")
            t_uz = sb.tile([128, 1024], dt, tag="u_z")
            # warmup tiles are zeroed first on GpSimd (earliest-ready queue),
            # then misc128 streams via SWDGE while the SP-queue preamble is
            # still draining; weights stream on the SP queue split into
            # pieces matching PE consumption order so each piece's semaphore
            # releases its matmuls as early as possible
            t_wl = sb.tile([128, B], dt, tag="warm_l")
            t_wr = sb.tile([128, 64], dt, tag="warm_r")
            nc.gpsimd.memset(t_wl[:], 0.0)
            nc.gpsimd.memset(t_wr[:], 0.0)
            nc.sync.dma_start(t_misc128[:], d_misc128[:])
            nc.sync.dma_start(t_w[:, 0:768], d_w[:, 0:768])
            nc.sync.dma_start(t_w[:, 768:1536], d_w[:, 768:1536])
            nc.sync.dma_start(t_urn[:, 0:1024], d_u_rn[:, 0:1024])
            nc.sync.dma_start(t_urn[:, 1024:2048], d_u_rn[:, 1024:2048])
            nc.sync.dma_start(t_uz[:, 0:768], d_u_z[:, 0:768])
            nc.sync.dma_start(t_uz[:, 768:1024], d_u_z[:, 768:1024])
            nc.scalar.dma_start(t_misc32[:], d_misc32[:])
            nc.scalar.dma_start(t_sel[:], d_sel[:])

            def inpT(c4):
                return t_misc128[:, _M128_INPT + c4 * B:_M128_INPT + (c4 + 1) * B]

            def hxT(c8):
                return t_misc128[:, _M128_HXT + c8 * B:_M128_HXT + (c8 + 1) * B]

            # PE warmup: small data-independent matmuls fill the PE-idle
            # window before routing so the HAM clock ramps early
            t_ones = sb.tile([1, B], dt, tag="ones")
            nc.gpsimd.memset(t_ones[:], 1.0)
            ps_warm = ps.tile([B, 64], dt, tag="ps_warm")
            for _ in range(_N_WARMUP):
                nc.tensor.matmul(ps_warm[:], t_wl[:], t_wr[:], start=True, stop=True)

            # routing logits: BETA*(input @ w_ik + hx @ w_hk + b_ik) -> [B, K]
            # (BETA folded into the weights host-side; bias via ones-row matmul)
            ps_ky = ps.tile([B, K], dt, tag="ps_ky")
            for c4 in range(4):
                nc.tensor.matmul(
                    ps_ky[:], inpT(c4),
                    t_misc128[:, _M128_WIK + c4 * K:_M128_WIK + (c4 + 1) * K],
                    start=(c4 == 0), stop=False)
            for c8 in range(8):
                nc.tensor.matmul(
                    ps_ky[:], hxT(c8),
                    t_misc128[:, _M128_WHK + c8 * K:_M128_WHK + (c8 + 1) * K],
                    start=False, stop=False)
            nc.tensor.matmul(ps_ky[:], t_ones[:],
                             t_misc32[0:1, _M32_BIK:_M32_BIK + K],
                             start=False, stop=True)

            # softmax over the K=4 free axis, normalized result lands in the
            # zero-padded [32, 32] tile that feeds the transpose
            t_kypad = sb.tile([B, B], dt, tag="kypad")
            nc.gpsimd.memset(t_kypad[:], 0.0)
            t_nmx = sb.tile([B, 1], dt, tag="nmx")
            nc.vector.reduce_max(t_nmx[:], ps_ky[:], axis=AX.X, negate=True)
            t_e = sb.tile([B, K], dt, tag="e")
            nc.scalar.activation(t_e[:], ps_ky[:], AF.Exp, bias=t_nmx[:])
            t_s = sb.tile([B, 1], dt, tag="s")
            nc.vector.reduce_sum(t_s[:], t_e[:], axis=AX.X)
            t_rs = sb.tile([B, 1], dt, tag="rs")
            nc.vector.reciprocal(t_rs[:], t_s[:])
            nc.vector.tensor_scalar_mul(t_kypad[:, 0:K], t_e[:], t_rs[:])
            t_kyT = sb.tile([B, B], dt, tag="kyT")
            nc.vector.transpose(t_kyT[:], t_kypad[:])

            # ky lands in the output tile early (off the critical tail)
            t_out = sb.tile([B, HS + K], dt, tag="out")
            nc.vector.tensor_copy(t_out[:, HS:HS + K], t_kypad[:, 0:K])

            # i-path: G_i = input @ Wm.T  -> [B, 384]
            ps_i = ps.tile([B, 3 * HS], dt, tag="ps_i")
            for c4 in range(4):
                nc.tensor.matmul(ps_i[:], inpT(c4),
                                 t_w[:, c4 * 384:(c4 + 1) * 384],
                                 start=(c4 == 0), stop=(c4 == 3))

            # gather the needed ky / kx columns via one-hot matmuls
            ps_g = ps.tile([B, 10], dt, tag="ps_g")
            nc.tensor.matmul(ps_g[:], t_kyT[0:K, :],
                             t_sel[0:K, _SEL_KY:_SEL_KY + 10], start=True, stop=True)
            ps_gx = ps.tile([B, 4], dt, tag="ps_gx")
            nc.tensor.matmul(ps_gx[:], t_sel[0:K, _SEL_KXT:_SEL_KXT + B],
                             t_sel[0:K, _SEL_KX:_SEL_KX + 4], start=True, stop=True)
            t_g = sb.tile([B, 14], dt, tag="g")
            nc.vector.tensor_copy(t_g[:, 0:10], ps_g[:])
            nc.vector.tensor_copy(t_g[:, 10:14], ps_gx[:])

            # kk products per 64-wide segment: kx[:,k1(g)] * ky[:,k2(g,seg)]
            # (n-gate entries carry the folded 0.5 via sel)
            t_kkp = sb.tile([B, 6], dt, tag="kkp")
            for j in range(6):
                g = j // 2
                nc.vector.tensor_mul(t_kkp[:, j:j + 1], t_g[:, 3 + j:4 + j],
                                     t_g[:, 10 + g:11 + g])
            # 0.5 * hx * kx_h, needed only at the very end — compute early
            t_u2h = sb.tile([B, HS], dt, tag="u2h")
            nc.vector.tensor_scalar_mul(t_u2h[:], t_misc32[:, _M32_HX:_M32_HX + HS],
                                        t_g[:, 13:14])

            # ib = ky_g * G_i + bias   (bias = bsum for z/r, bias_ih for n)
            t_ib = sb.tile([B, 3 * HS], dt, tag="ib")
            for g, boff in ((0, _M32_BSZ), (1, _M32_BSR), (2, _M32_BIN)):
                nc.vector.scalar_tensor_tensor(
                    t_ib[:, g * HS:(g + 1) * HS], ps_i[:, g * HS:(g + 1) * HS],
                    t_g[:, g:g + 1], t_misc32[:, boff:boff + HS],
                    op0=OP.mult, op1=OP.add)

            # h-path: r|n gates first (they gate the long tail), z last
            ps_rn = ps.tile([B, 2 * HS], dt, tag="ps_rn")
            for c8 in range(8):
                nc.tensor.matmul(ps_rn[:], hxT(c8),
                                 t_urn[:, c8 * 256:(c8 + 1) * 256],
                                 start=(c8 == 0), stop=(c8 == 7))
            ps_z = ps.tile([B, HS], dt, tag="ps_z")
            for c8 in range(8):
                nc.tensor.matmul(ps_z[:], hxT(c8),
                                 t_uz[:, c8 * HS:(c8 + 1) * HS],
                                 start=(c8 == 0), stop=(c8 == 7))

            # S = kk * G_h + ib per 64-segment
            t_Sr = sb.tile([B, HS], dt, tag="Sr")
            t_hn = sb.tile([B, HS], dt, tag="hn")   # 0.5*(kk*G_h_n + bias_hh_n)
            t_Sz = sb.tile([B, HS], dt, tag="Sz")
            for si in range(2):
                s0 = si * 64
                nc.vector.scalar_tensor_tensor(
                    t_Sr[:, s0:s0 + 64], ps_rn[:, s0:s0 + 64],
                    t_kkp[:, 2 + si:3 + si], t_ib[:, HS + s0:HS + s0 + 64],
                    op0=OP.mult, op1=OP.add)
            for si in range(2):
                s0 = si * 64
                nc.vector.scalar_tensor_tensor(
                    t_hn[:, s0:s0 + 64], ps_rn[:, HS + s0:HS + s0 + 64],
                    t_kkp[:, 4 + si:5 + si],
                    t_misc32[:, _M32_BHN + s0:_M32_BHN + s0 + 64],
                    op0=OP.mult, op1=OP.add)

            # sigmoid via tanh: r = 0.5*th_r + 0.5, and r*h_n = (th_r+1)*hn
            t_thr = sb.tile([B, HS], dt, tag="thr")
            nc.scalar.activation(t_thr[:], t_Sr[:], AF.Tanh, scale=0.5)
            t_hnr = sb.tile([B, HS], dt, tag="hnr")
            nc.vector.scalar_tensor_tensor(t_hnr[:], t_thr[:], 1.0, t_hn[:],
                                           op0=OP.add, op1=OP.mult)
            t_pren = sb.tile([B, HS], dt, tag="pren")
            nc.vector.tensor_add(t_pren[:], t_ib[:, 2 * HS:3 * HS], t_hnr[:])
            t_newg = sb.tile([B, HS], dt, tag="newg")
            nc.scalar.activation(t_newg[:], t_pren[:], AF.Tanh)

            for si in range(2):
                s0 = si * 64
                nc.vector.scalar_tensor_tensor(
                    t_Sz[:, s0:s0 + 64], ps_z[:, s0:s0 + 64],
                    t_kkp[:, si:si + 1], t_ib[:, s0:s0 + 64],
                    op0=OP.mult, op1=OP.add)
            t_thz = sb.tile([B, HS], dt, tag="thz")
            nc.scalar.activation(t_thz[:], t_Sz[:], AF.Tanh, scale=0.5)

            # hy = 2*t1h + (th_z+1)*d0h,  t1h = 0.5*n*ky_h,  d0h = u2h - t1h
            t_t1h = sb.tile([B, HS], dt, tag="t1h")
            nc.vector.tensor_scalar_mul(t_t1h[:], t_newg[:], t_g[:, 9:10])
            t_d0h = sb.tile([B, HS], dt, tag="d0h")
            nc.vector.tensor_sub(t_d0h[:], t_u2h[:], t_t1h[:])
            t_e2 = sb.tile([B, HS], dt, tag="e2")
            nc.vector.scalar_tensor_tensor(t_e2[:], t_thz[:], 1.0, t_d0h[:],
                                           op0=OP.add, op1=OP.mult)
            nc.vector.scalar_tensor_tensor(t_out[:, 0:HS], t_t1h[:], 2.0, t_e2[:],
                                           op0=OP.mult, op1=OP.add)
            nc.sync.dma_start(d_out[:], t_out[:])

    nc.finalize()
    return nc


_NC = None


def _get_nc():
    global _NC
    if _NC is None:
        _NC = build_nc()
    return _NC


def kernel(**inputs):
    from concourse.bass_utils import run_bass_kernel_spmd

    nc = _get_nc()
    in_maps = prep_inputs(**{k: np.asarray(v) for k, v in inputs.items()})
    res = run_bass_kernel_spmd(nc, in_maps, list(range(NCORES)))
    outs = res.results
    hy = np.concatenate([outs[m]["out"][:, 0:HS] for m in range(NCORES)], axis=1)
    ky = np.ascontiguousarray(outs[0]["out"][:, HS:HS + K])
    return hy, ky


# revision 7
# speedup vs baseline: 1.1940x; 1.1242x over previous
"""Trainium2 Bass kernel for BSGRUCell (block-sparse GRU cell with soft MoE routing).

Math (algebraically equivalent to the reference, verified):
    ky  = softmax(BETA * (input @ w_ik + hx @ w_hk + b_ik))            [B, K]
    G_i = input @ W2d.T          (W2d = weight_ih.reshape(3H, IN))     [B, 3H]
    G_h = hx    @ U2d.T          (U2d = weight_hh.reshape(3H, H))      [B, 3H]
    i_gates = G_i * ky[:, o//768]                       + bias_ih
    h_gates = G_h * kx[:, o//768] * ky[:, (o%768)//192] + bias_hh
    z, r = sigmoid(i_z + h_z), sigmoid(i_r + h_r)
    n    = tanh(i_n + h_n * r)
    hy   = n * ky_h + z * (hx * kx_h - n * ky_h),  ky_h/kx_h = col h//256

Sharding: the 3H=3072 gate-output rows are split by hidden slice — core m owns
hidden columns [m*128, (m+1)*128) and the three gate rows g*1024 + m*128 for
g in z,r,n.  No cross-core communication; weights are read exactly once across
the 8 cores (memory-roofline sharding).  All per-core index variation (which
ky/kx columns scale which output segments) is passed as small one-hot selection
matrices so a single SPMD program serves all cores.

Device-side structure per core:
  - small data-independent warmup matmuls get the PE HAM clock to 8/8 before
    the real fp32 matmuls arrive (fp32 is two PE passes per matmul)
  - BETA is folded into the routing weights on the host; the routing bias is
    added via a K=1 ones-row matmul so the softmax needs no pre-add
  - sigmoid(x) is computed as 0.5*tanh(0.5x)+0.5 so exp/tanh share one ACT
    table set; the 0.5/2.0 factors are folded into the one-hot gather matrices
    and biases on the host, so no extra affine ops appear on the critical tail
  - U streams in PE-consumption order (r|n gate columns first, z last) so the
    post-DMA tail is short; the z gate only feeds the final two ops
"""

import numpy as np

B = 32
IN = 512
H = 1024
K = 4
BS = H // K          # 256
H3 = 3 * H           # 3072
NCORES = 8
HS = H // NCORES     # 128
BETA = 10.0

# misc32 column layout
_M32_HX = 0          # [0:128)    hx[:, m*128:(m+1)*128]
_M32_BSZ = 128       # [128:256)  (bias_ih+bias_hh) z-slice, tiled to 32 rows
_M32_BSR = 256       # [256:384)  (bias_ih+bias_hh) r-slice
_M32_BIN = 384       # [384:512)  bias_ih n-slice
_M32_BHN = 512       # [512:640)  0.5 * bias_hh n-slice
_M32_BIK = 640       # [640:644)  BETA * bias_ik tiled
_M32_W = 644

# misc128 column layout
_M128_INPT = 0       # [0:128)    input.T packed   [p, c4*32+b]
_M128_HXT = 128      # [128:384)  hx.T packed      [p, c8*32+b]
_M128_WIK = 384      # [384:400)  BETA*weight_ik packed [p, c4*4+k]
_M128_WHK = 400      # [400:432)  BETA*weight_hk packed [p, c8*4+k]
_M128_W = 432

# sel column layout: 10 ky one-hots, 4 kx one-hots, kx.T
# ky cols: [ki_z, ki_r, ki_n, k2_z0, k2_z1, k2_r0, k2_r1, k2_n0*, k2_n1*, kh*]
# kx cols: [k1_z, k1_r, k1_n, kh*]        (* = scaled by 0.5, see module doc)
_SEL_KY = 0          # [0:10)
_SEL_KX = 10         # [10:14)
_SEL_KXT = 14        # [14:46)
_SEL_W = 46

_N_WARMUP = 15


def _pack_rows(x, nchunk):
    """[nchunk*128, F] -> [128, nchunk*F] with col block c = rows [c*128,(c+1)*128)."""
    f = x.shape[1]
    return np.ascontiguousarray(
        x.reshape(nchunk, 128, f).transpose(1, 0, 2).reshape(128, nchunk * f)
    )


def prep_inputs(input, hx, kx, weight_ik, weight_hk, weight_ih, weight_hh,
                bias_ik, bias_ih, bias_hh):
    """Build the 8 per-core input maps (all float32 numpy)."""
    f32 = np.float32
    input = np.asarray(input, f32)
    hx = np.asarray(hx, f32)
    kx = np.asarray(kx, f32)
    weight_ik = np.asarray(weight_ik, f32)
    weight_hk = np.asarray(weight_hk, f32)
    W2d = np.asarray(weight_ih, f32).reshape(H3, IN)
    U2d = np.asarray(weight_hh, f32).reshape(H3, H)
    bias_ik = np.asarray(bias_ik, f32)
    bias_ih = np.asarray(bias_ih, f32)
    bias_hh = np.asarray(bias_hh, f32)
    bsum = bias_ih + bias_hh

    misc128 = np.empty((128, _M128_W), f32)
    misc128[:, _M128_INPT:_M128_INPT + 128] = _pack_rows(input.T, 4)
    misc128[:, _M128_HXT:_M128_HXT + 256] = _pack_rows(hx.T, 8)
    misc128[:, _M128_WIK:_M128_WIK + 16] = _pack_rows(BETA * weight_ik, 4)
    misc128[:, _M128_WHK:_M128_WHK + 32] = _pack_rows(BETA * weight_hk, 8)

    in_maps = []
    for m in range(NCORES):
        og = [g * H + m * HS for g in range(3)]

        misc32 = np.empty((B, _M32_W), f32)
        misc32[:, _M32_HX:_M32_HX + HS] = hx[:, m * HS:(m + 1) * HS]
        misc32[:, _M32_BSZ:_M32_BSZ + HS] = bsum[og[0]:og[0] + HS][None, :]
        misc32[:, _M32_BSR:_M32_BSR + HS] = bsum[og[1]:og[1] + HS][None, :]
        misc32[:, _M32_BIN:_M32_BIN + HS] = bias_ih[og[2]:og[2] + HS][None, :]
        misc32[:, _M32_BHN:_M32_BHN + HS] = 0.5 * bias_hh[og[2]:og[2] + HS][None, :]
        misc32[:, _M32_BIK:_M32_BIK + K] = BETA * bias_ik[None, :]

        sel = np.zeros((K, _SEL_W), f32)
        # ky gathers; n-gate k2 entries and kh carry the folded 0.5
        idx_ky = [og[0] // 768, og[1] // 768, og[2] // 768]
        for g in range(3):
            a = og[g] % 768
            idx_ky += [a // 192, (a + 64) // 192]
        idx_ky.append(m // 2)
        for j, k in enumerate(idx_ky):
            sel[k, _SEL_KY + j] = 0.5 if j in (7, 8, 9) else 1.0
        # kx gathers: [k1_z, k1_r, k1_n, 0.5*kh]
        idx_kx = [og[0] // 768, og[1] // 768, og[2] // 768, m // 2]
        for j, k in enumerate(idx_kx):
            sel[k, _SEL_KX + j] = 0.5 if j == 3 else 1.0
        sel[:, _SEL_KXT:_SEL_KXT + B] = kx.T

        rows = np.concatenate([np.arange(o, o + HS) for o in og])
        w = _pack_rows(np.ascontiguousarray(W2d[rows].T), 4)        # [128, 1536]
        Um = U2d[rows]                                              # [384, 1024]
        u_rn = _pack_rows(np.ascontiguousarray(Um[HS:3 * HS].T), 8)  # [128, 2048]
        u_z = _pack_rows(np.ascontiguousarray(Um[0:HS].T), 8)        # [128, 1024]

        in_maps.append({
            "misc128": misc128, "misc32": misc32, "sel": sel, "w": w,
            "u_rn": u_rn, "u_z": u_z,
        })
    return in_maps


def build_nc():
    import concourse.bacc as bacc
    import concourse.bass as bass
    import concourse.tile as tile
    from concourse import mybir

    dt = mybir.dt.float32
    AF = mybir.ActivationFunctionType
    OP = mybir.AluOpType
    AX = mybir.AxisListType

    nc = bacc.Bacc("TRN2")
    d_misc128 = nc.dram_tensor("misc128", [128, _M128_W], dt, kind="ExternalInput")
    d_misc32 = nc.dram_tensor("misc32", [B, _M32_W], dt, kind="ExternalInput")
    d_sel = nc.dram_tensor("sel", [K, _SEL_W], dt, kind="ExternalInput")
    d_w = nc.dram_tensor("w", [128, 1536], dt, kind="ExternalInput")
    d_u_rn = nc.dram_tensor("u_rn", [128, 2048], dt, kind="ExternalInput")
    d_u_z = nc.dram_tensor("u_z", [128, 1024], dt, kind="ExternalInput")
    d_out = nc.dram_tensor("out", [B, HS + K], dt, kind="ExternalOutput")

    with tile.TileContext(nc) as tc:
        with (
            tc.tile_pool(name="sb", bufs=1) as sb,
            tc.tile_pool(name="ps", bufs=1, space=bass.MemorySpace.PSUM) as ps,
        ):
            t_misc128 = sb.tile([128, _M128_W], dt, tag="misc128")
            t_misc32 = sb.tile([B, _M32_W], dt, tag="misc32")
            t_sel = sb.tile([K, _SEL_W], dt, tag="sel")
            t_w = sb.tile([128, 1536], dt, tag="w")
            t_urn = sb.tile([128, 2048], dt, tag="u_rn")
            t_uz = sb.tile([128, 1024], dt, tag="u_z")
            # warmup tiles are zeroed first on GpSimd (earliest-ready queue),
            # then misc128 streams via SWDGE while the SP-queue preamble is
            # still draining; weights stream on the SP queue split into
            # pieces matching PE consumption order so each piece's semaphore
            # releases its matmuls as early as possible
            t_wl = sb.tile([128, B], dt, tag="warm_l")
            t_wr = sb.tile([128, 64], dt, tag="warm_r")
            nc.gpsimd.memset(t_wl[:], 0.0)
            nc.gpsimd.memset(t_wr[:], 0.0)
            nc.sync.dma_start(t_misc128[:], d_misc128[:])
            nc.sync.dma_start(t_w[:, 0:768], d_w[:, 0:768])
            nc.sync.dma_start(t_w[:, 768:1536], d_w[:, 768:1536])
            nc.sync.dma_start(t_urn[:, 0:1024], d_u_rn[:, 0:1024])
            nc.sync.dma_start(t_urn[:, 1024:2048], d_u_rn[:, 1024:2048])
            nc.sync.dma_start(t_uz[:, 0:768], d_u_z[:, 0:768])
            nc.sync.dma_start(t_uz[:, 768:1024], d_u_z[:, 768:1024])
            nc.scalar.dma_start(t_misc32[:], d_misc32[:])
            nc.scalar.dma_start(t_sel[:], d_sel[:])

            def inpT(c4):
                return t_misc128[:, _M128_INPT + c4 * B:_M128_INPT + (c4 + 1) * B]

            def hxT(c8):
                return t_misc128[:, _M128_HXT + c8 * B:_M128_HXT + (c8 + 1) * B]

            # PE warmup: small data-independent matmuls fill the PE-idle
            # window before routing so the HAM clock ramps early
            t_ones = sb.tile([1, B], dt, tag="ones")
            nc.gpsimd.memset(t_ones[:], 1.0)
            ps_warm = ps.tile([B, 64], dt, tag="ps_warm")
            for _ in range(_N_WARMUP):
                nc.tensor.matmul(ps_warm[:], t_wl[:], t_wr[:], start=True, stop=True)

            # routing logits: BETA*(input @ w_ik + hx @ w_hk + b_ik) -> [B, K]
            # (BETA folded into the weights host-side; bias via ones-row matmul)
            ps_ky = ps.tile([B, K], dt, tag="ps_ky")
            for c4 in range(4):
                nc.tensor.matmul(
                    ps_ky[:], inpT(c4),
                    t_misc128[:, _M128_WIK + c4 * K:_M128_WIK + (c4 + 1) * K],
                    start=(c4 == 0), stop=False)
            for c8 in range(8):
                nc.tensor.matmul(
                    ps_ky[:], hxT(c8),
                    t_misc128[:, _M128_WHK + c8 * K:_M128_WHK + (c8 + 1) * K],
                    start=False, stop=False)
            nc.tensor.matmul(ps_ky[:], t_ones[:],
                             t_misc32[0:1, _M32_BIK:_M32_BIK + K],
                             start=False, stop=True)

            # softmax over the K=4 free axis, normalized result lands in the
            # zero-padded [32, 32] tile that feeds the transpose
            t_kypad = sb.tile([B, B], dt, tag="kypad")
            nc.gpsimd.memset(t_kypad[:], 0.0)
            t_nmx = sb.tile([B, 1], dt, tag="nmx")
            nc.vector.reduce_max(t_nmx[:], ps_ky[:], axis=AX.X, negate=True)
            t_e = sb.tile([B, K], dt, tag="e")
            nc.scalar.activation(t_e[:], ps_ky[:], AF.Exp, bias=t_nmx[:])
            t_s = sb.tile([B, 1], dt, tag="s")
            nc.vector.reduce_sum(t_s[:], t_e[:], axis=AX.X)
            t_rs = sb.tile([B, 1], dt, tag="rs")
            nc.vector.reciprocal(t_rs[:], t_s[:])
            nc.vector.tensor_scalar_mul(t_kypad[:, 0:K], t_e[:], t_rs[:])
            t_kyT = sb.tile([B, B], dt, tag="kyT")
            nc.vector.transpose(t_kyT[:], t_kypad[:])

            # ky lands in the output tile early (off the critical tail)
            t_out = sb.tile([B, HS + K], dt, tag="out")
            nc.vector.tensor_copy(t_out[:, HS:HS + K], t_kypad[:, 0:K])

            # i-path: G_i = input @ Wm.T  -> [B, 384]
            ps_i = ps.tile([B, 3 * HS], dt, tag="ps_i")
            for c4 in range(4):
                nc.tensor.matmul(ps_i[:], inpT(c4),
                                 t_w[:, c4 * 384:(c4 + 1) * 384],
                                 start=(c4 == 0), stop=(c4 == 3))

            # gather the needed ky / kx columns via one-hot matmuls
            ps_g = ps.tile([B, 10], dt, tag="ps_g")
            nc.tensor.matmul(ps_g[:], t_kyT[0:K, :],
                             t_sel[0:K, _SEL_KY:_SEL_KY + 10], start=True, stop=True)
            ps_gx = ps.tile([B, 4], dt, tag="ps_gx")
            nc.tensor.matmul(ps_gx[:], t_sel[0:K, _SEL_KXT:_SEL_KXT + B],
                             t_sel[0:K, _SEL_KX:_SEL_KX + 4], start=True, stop=True)
            t_g = sb.tile([B, 14], dt, tag="g")
            nc.vector.tensor_copy(t_g[:, 0:10], ps_g[:])
            nc.vector.tensor_copy(t_g[:, 10:14], ps_gx[:])

            # kk products per 64-wide segment: kx[:,k1(g)] * ky[:,k2(g,seg)]
            # (n-gate entries carry the folded 0.5 via sel)
            t_kkp = sb.tile([B, 6], dt, tag="kkp")
            for j in range(6):
                g = j // 2
                nc.vector.tensor_mul(t_kkp[:, j:j + 1], t_g[:, 3 + j:4 + j],
                                     t_g[:, 10 + g:11 + g])
            # 0.5 * hx * kx_h, needed only at the very end — compute early
            t_u2h = sb.tile([B, HS], dt, tag="u2h")
            nc.vector.tensor_scalar_mul(t_u2h[:], t_misc32[:, _M32_HX:_M32_HX + HS],
                                        t_g[:, 13:14])

            # ib = ky_g * G_i + bias   (bias = bsum for z/r, bias_ih for n)
            t_ib = sb.tile([B, 3 * HS], dt, tag="ib")
            for g, boff in ((0, _M32_BSZ), (1, _M32_BSR), (2, _M32_BIN)):
                nc.vector.scalar_tensor_tensor(
                    t_ib[:, g * HS:(g + 1) * HS], ps_i[:, g * HS:(g + 1) * HS],
                    t_g[:, g:g + 1], t_misc32[:, boff:boff + HS],
                    op0=OP.mult, op1=OP.add)

            # h-path: r|n gates first (they gate the long tail), z last
            ps_rn = ps.tile([B, 2 * HS], dt, tag="ps_rn")
            for c8 in range(8):
                nc.tensor.matmul(ps_rn[:], hxT(c8),
                                 t_urn[:, c8 * 256:(c8 + 1) * 256],
                                 start=(c8 == 0), stop=(c8 == 7))
            ps_z = ps.tile([B, HS], dt, tag="ps_z")
            for c8 in range(8):
                nc.tensor.matmul(ps_z[:], hxT(c8),
                                 t_uz[:, c8 * HS:(c8 + 1) * HS],
                                 start=(c8 == 0), stop=(c8 == 7))

            # S = kk * G_h + ib per 64-segment
            t_Sr = sb.tile([B, HS], dt, tag="Sr")
            t_hn = sb.tile([B, HS], dt, tag="hn")   # 0.5*(kk*G_h_n + bias_hh_n)
            t_Sz = sb.tile([B, HS], dt, tag="Sz")
            for si in range(2):
                s0 = si * 64
                nc.vector.scalar_tensor_tensor(
                    t_Sr[:, s0:s0 + 64], ps_rn[:, s0:s0 + 64],
                    t_kkp[:, 2 + si:3 + si], t_ib[:, HS + s0:HS + s0 + 64],
                    op0=OP.mult, op1=OP.add)
            for si in range(2):
                s0 = si * 64
                nc.vector.scalar_tensor_tensor(
                    t_hn[:, s0:s0 + 64], ps_rn[:, HS + s0:HS + s0 + 64],
                    t_kkp[:, 4 + si:5 + si],
                    t_misc32[:, _M32_BHN + s0:_M32_BHN + s0 + 64],
                    op0=OP.mult, op1=OP.add)

            # sigmoid via tanh: r = 0.5*th_r + 0.5, and r*h_n = (th_r+1)*hn
            t_thr = sb.tile([B, HS], dt, tag="thr")
            nc.scalar.activation(t_thr[:], t_Sr[:], AF.Tanh, scale=0.5)
            t_hnr = sb.tile([B, HS], dt, tag="hnr")
            nc.vector.scalar_tensor_tensor(t_hnr[:], t_thr[:], 1.0, t_hn[:],
                                           op0=OP.add, op1=OP.mult)
            t_pren = sb.tile([B, HS], dt, tag="pren")
            nc.vector.tensor_add(t_pren[:], t_ib[:, 2 * HS:3 * HS], t_hnr[:])
            t_newg = sb.tile([B, HS], dt, tag="newg")
            nc.scalar.activation(t_newg[:], t_pren[:], AF.Tanh)

            for si in range(2):
                s0 = si * 64
                nc.vector.scalar_tensor_tensor(
                    t_Sz[:, s0:s0 + 64], ps_z[:, s0:s0 + 64],
                    t_kkp[:, si:si + 1], t_ib[:, s0:s0 + 64],
                    op0=OP.mult, op1=OP.add)
            t_thz = sb.tile([B, HS], dt, tag="thz")
            nc.scalar.activation(t_thz[:], t_Sz[:], AF.Tanh, scale=0.5)

            # hy = 2*t1h + (th_z+1)*d0h,  t1h = 0.5*n*ky_h,  d0h = u2h - t1h
            t_t1h = sb.tile([B, HS], dt, tag="t1h")
            nc.vector.tensor_scalar_mul(t_t1h[:], t_newg[:], t_g[:, 9:10])
            t_d0h = sb.tile([B, HS], dt, tag="d0h")
            nc.vector.tensor_sub(t_d0h[:], t_u2h[:], t_t1h[:])
            t_e2 = sb.tile([B, HS], dt, tag="e2")
            nc.vector.scalar_tensor_tensor(t_e2[:], t_thz[:], 1.0, t_d0h[:],
                                           op0=OP.add, op1=OP.mult)
            nc.vector.scalar_tensor_tensor(t_out[:, 0:HS], t_t1h[:], 2.0, t_e2[:],
                                           op0=OP.mult, op1=OP.add)
            nc.sync.dma_start(d_out[:], t_out[:])

    nc.finalize()
    return nc


_NC = None


def _get_nc():
    global _NC
    if _NC is None:
        _NC = build_nc()
    return _NC


def kernel(**inputs):
    from concourse.bass_utils import run_bass_kernel_spmd

    nc = _get_nc()
    in_maps = prep_inputs(**{k: np.asarray(v) for k, v in inputs.items()})
    res = run_bass_kernel_spmd(nc, in_maps, list(range(NCORES)))
    outs = res.results
    hy = np.concatenate([outs[m]["out"][:, 0:HS] for m in range(NCORES)], axis=1)
    ky = np.ascontiguousarray(outs[0]["out"][:, HS:HS + K])
    return hy, ky
